# revision 1
# baseline (speedup 1.0000x reference)
"""Trainium2 Bass kernel for nn_ContConv1dDenseSim (banded continuous conv).

Math (reference):
  dt[b,l,j] = times[b,l]-times[b,j], masked to a causal band j in [l-W+1, l]
  (W = (sim_size+1)*kernel_size = 30), true_ids[b,j], and a row-validity mask.
  h = relu(dt*w1+b1)  (8 hidden), kv = (h@w2+b2) masked, reshaped (16,16)
  out[b,l,o] = sum_{j,i} features[b,j,i] * kv[b,l,j,i,o]

Factorization used here:
  G[b,j,k,o]  = sum_i f[b,j,i] * W2[k,i,o]   (k=0..7), G[b,j,8,o] = f[b,j]@B2
  A_k[j,l]    = band(l-j) * relu(dt[l,j]*w1[k]+b1[k])  (k=0..7), A_8 = band
  out[b,l,o]  = row_valid[l] * sum_{j,k} A_k[j,l] * (true_ids[j]*G[b,j,k,o])

Sharding: 8 cores = 2 batches x 4 query-row blocks of 128. Each core sees a
157-column window (128 + W-1) of keys and produces a (128,16) output block
(returned transposed as (16,128); the host transposes back).

On-device layout: window column index jl on SBUF partitions, query row p on
the free dim. The 157-long window is stored as a 256-wide "folded" pair of
column blocks [jl=0..127 | jl=128..156 (+pad)] sharing partitions, so the
relu/mask stages process one (128,256) tile per hidden channel. The banded
contraction is 18 PSUM-accumulated matmuls (9 channels x 2 K-splits) with the
small G factor stationary.

NOTE: TRN2 engine instructions only encode a single sync-wait, so the program
is ordered so each engine's first touch of any foreign-produced tensor is an
instruction with exactly one new cross-engine dependency (tiny "observer" ops
where needed), and the Tile kernel-tail drain is pre-satisfied by single-wait
SP nops.
"""

import numpy as np
import concourse.bass as bass
import concourse.tile as tile
import concourse.mybir as mybir
from concourse.bass_utils import run_bass_kernel_spmd
from concourse.tile_rust import add_dep_helper

F32 = mybir.dt.float32
Alu = mybir.AluOpType
Act = mybir.ActivationFunctionType

BS, L, CH, HID, KS = 2, 512, 16, 8, 5
LBLK = 128                      # query rows per core
NBLK = L // LBLK                # 4
NCORES = BS * NBLK              # 8
NKP = HID + 1                   # A channels (8 hidden + mask)
NF = NKP * CH                   # 144 G columns
W2 = 2 * LBLK                   # folded window width (256)
NPAR = 3 + 2 * HID              # packed per-partition params columns

# test harness hooks
TRACE = False
LAST = None

_prog_cache = {}


def _build(W):
    """Build the single-core SPMD program for band width W (30 for sim=5)."""
    WIN = LBLK + W - 1          # real window columns (157)
    LO = WIN - 128              # columns in the second fold (29)
    nc = bass.Bass(trn_type="TRN2")

    # [ones; t_win padded to 256] (cols 0:256) | [t_row; -ones] (cols 256:384)
    dtpk = nc.declare_dram_parameter("dtpk", [2, W2 + LBLK], F32,
                                     isOutput=False)
    # feat_win^T padded to 256 | W2p with b2 block (cols 256:400)
    fw = nc.declare_dram_parameter("fw", [CH, W2 + NF], F32,
                                   isOutput=False)
    # col 0: tiw[0:128], col 1: tiw[128:WIN] (padded), col 2: row_valid,
    # cols 3:3+HID: w1 replicated, cols 3+HID:3+2*HID: b1 replicated
    par = nc.declare_dram_parameter("par", [128, NPAR], F32, isOutput=False)
    out_d = nc.declare_dram_parameter("out", [LBLK, CH], F32, isOutput=True)

    with tile.TileContext(nc) as tc:
        with (
            tc.tile_pool(name="sb", bufs=1) as sb,
            tc.tile_pool(name="ps", bufs=1, space="PSUM") as ps,
        ):
            # ---- input loads: two issuing sequencers, one DMA per group ----
            t_dtpk = sb.tile([2, W2 + LBLK], F32)
            dma_a = nc.sync.dma_start(t_dtpk[:], dtpk[:])
            t_fw = sb.tile([CH, W2 + NF], F32)
            dma_b = nc.scalar.dma_start(t_fw[:], fw[:])
            t_par = sb.tile([128, NPAR], F32)
            dma_c = nc.sync.dma_start(t_par[:], par[:])
            tiw_up = t_par[:, 0:1]
            tiw_lo = t_par[0:LO, 1:2]
            rv = t_par[:, 2:3]

            # ---- band mask, folded: [:,0:128] up block, [:,128:256] lo ----
            ones = sb.tile([128, W2], F32)
            nc.vector.memset(ones[:], 1.0)
            btmp = sb.tile([128, W2], F32)
            band = sb.tile([128, W2], F32)
            # up: keep jl - p >= 0  (jl = q)
            nc.gpsimd.affine_select(btmp[:, 0:LBLK], ones[:, 0:LBLK],
                                    [[-1, LBLK]], Alu.is_ge, 0.0,
                                    base=0, channel_multiplier=1)
            # up: keep (W-1) - jl + p >= 0
            nc.gpsimd.affine_select(band[:, 0:LBLK], btmp[:, 0:LBLK],
                                    [[1, LBLK]], Alu.is_ge, 0.0,
                                    base=W - 1, channel_multiplier=-1)
            # lo (jl = 128+q): keep p - (128-(W-1)) - q >= 0
            nc.gpsimd.affine_select(btmp[:, LBLK:W2], ones[:, LBLK:W2],
                                    [[1, LBLK]], Alu.is_ge, 0.0,
                                    base=(W - 1) - 128, channel_multiplier=-1)
            # lo: keep (LO-1) - q >= 0  (zero the fold's padding rows)
            last_gp = nc.gpsimd.affine_select(band[:, LBLK:W2],
                                              btmp[:, LBLK:W2],
                                              [[0, LBLK]], Alu.is_ge, 0.0,
                                              base=LO - 1,
                                              channel_multiplier=-1)

            # ---- observers (single-wait discipline, see module docstring) --
            obs_a = sb.tile([1, 1], F32)
            nc.scalar.activation(obs_a[:], t_par[0:1, 0:1], Act.Copy)
            obs_v = sb.tile([1, 2], F32)
            nc.vector.tensor_copy(obs_v[:, 0:1], t_par[0:1, 0:1])
            nc.vector.tensor_copy(obs_v[:, 1:2], band[0:1, LBLK:LBLK + 1])

            # ---- dtT[jl, p] = t_row[p] - t_win[jl], folded (128,256) ----
            p_dt = ps.tile([128, W2], F32)
            rhs_dt = t_dtpk[:, W2:W2 + LBLK]
            nc.tensor.matmul(p_dt[:, 0:LBLK], t_dtpk[:, 0:LBLK], rhs_dt,
                             start=True, stop=True)
            nc.tensor.matmul(p_dt[:, LBLK:W2], t_dtpk[:, LBLK:W2], rhs_dt,
                             start=True, stop=True)

            # ---- G[jl, k*16+o] = feat_win[jl] @ W2p, folded (128,288) ----
            p_g = ps.tile([128, 2 * NF], F32)
            w2p_s = t_fw[:, W2:W2 + NF]
            nc.tensor.matmul(p_g[:, 0:NF], t_fw[:, 0:LBLK],
                             w2p_s, start=True, stop=True)
            nc.tensor.matmul(p_g[:, NF:2 * NF], t_fw[:, LBLK:W2],
                             w2p_s, start=True, stop=True)
            g_sb = sb.tile([128, 2 * NF], F32)
            nc.vector.tensor_scalar_mul(g_sb[:, 0:NF], p_g[:, 0:NF], tiw_up)
            nc.vector.tensor_scalar_mul(g_sb[0:LO, NF:2 * NF],
                                        p_g[0:LO, NF:2 * NF], tiw_lo)

            # ---- A channels: relu(dt*w1k + b1k) * band, one (128,256)/k ----
            a_full = sb.tile([128, HID * W2], F32)
            last_act = None
            for k in range(HID):
                s = slice(k * W2, (k + 1) * W2)
                last_act = nc.scalar.activation(
                    a_full[:, s], p_dt[:], Act.Relu,
                    bias=t_par[:, 3 + HID + k:4 + HID + k],
                    scale=t_par[:, 3 + k:4 + k])
                nc.vector.tensor_mul(a_full[:, s], a_full[:, s], band[:])

            # ---- out[p, o] = sum_k sum_jl A_k[jl,p] * G[jl,k*16+o] ----
            # A-slices are the stationary side (fp32 LDWEIGHTS streams at
            # 2 cyc/row vs 4 cyc/row matmul); the 16-wide G is the moving
            # tensor, so each pair costs ~LDW only.
            p_out = ps.tile([LBLK, CH], F32)
            last_pe = None

            def a_lhs(k):
                if k < HID:
                    return a_full[:, k * W2:(k + 1) * W2]
                return band[:, :]

            for k in range(NKP):
                nc.tensor.matmul(p_out[:], a_lhs(k)[:, 0:LBLK],
                                 g_sb[:, k * CH:(k + 1) * CH],
                                 start=(k == 0), stop=False)
                last_pe = nc.tensor.matmul(
                    p_out[:], a_lhs(k)[0:LO, LBLK:W2],
                    g_sb[0:LO, NF + k * CH:NF + (k + 1) * CH],
                    start=False, stop=(k == NKP - 1))

            # ---- row-validity fold + store ----
            o_sb = sb.tile([LBLK, CH], F32)
            last_dve = nc.vector.tensor_scalar_mul(o_sb[:], p_out[:], rv)
            dma_o = nc.sync.dma_start(out_d[:], o_sb[:])

            # The Tile kernel-tail drain waits on every outstanding
            # semaphore, but TRN2 instructions encode at most one sync
            # wait. Observe each producer from the SP sequencer with
            # single-wait nops so the drain itself needs none.
            for prod in (dma_a, dma_b, dma_c, dma_o,
                         last_gp, last_act, last_dve, last_pe):
                nop = nc.sync.nop(nofuse=True, hint="predrain_observer")
                add_dep_helper(nop.ins, prod.ins, sync=True,
                               reason="pre-drain single-wait observer")

    heavy = [(nm, type(i).__name__, len(i.sync_info.on_wait))
             for nm, i in nc.inst_map.items()
             if getattr(i, "sync_info", None) is not None
             and i.sync_info.on_wait
             and len(i.sync_info.on_wait) > 1
             and type(i).__name__ != "InstDrain"]
    if heavy:
        raise RuntimeError(f"multi-wait instructions would fail walrus: {heavy}")
    return nc


def kernel(times, features, lengths, true_ids, sim_size, w1, b1, w2, b2):
    global LAST
    times = np.ascontiguousarray(np.asarray(times, dtype=np.float32))
    features = np.ascontiguousarray(np.asarray(features, dtype=np.float32))
    lengths = np.asarray(lengths)
    true_ids = np.asarray(true_ids)
    sim = int(np.asarray(sim_size))
    w1 = np.asarray(w1, dtype=np.float32).reshape(-1)
    b1 = np.asarray(b1, dtype=np.float32).reshape(-1)
    w2 = np.asarray(w2, dtype=np.float32)
    b2 = np.asarray(b2, dtype=np.float32)

    W = (sim + 1) * KS
    WIN = LBLK + W - 1
    LO = WIN - 128

    import os
    raw = bool(os.environ.get("BASS_RAW"))
    key = (W, raw)
    if key not in _prog_cache:
        if raw:
            import kernel_raw
            _prog_cache[key] = kernel_raw.build_raw(W)
        else:
            _prog_cache[key] = _build(W)
    nc = _prog_cache[key]

    w2p = np.concatenate(
        [w2.reshape(HID, CH, CH).transpose(1, 0, 2).reshape(CH, HID * CH),
         b2.reshape(CH, CH)], axis=1).astype(np.float32)

    in_maps = []
    for core in range(NCORES):
        b, blk = divmod(core, NBLK)
        l0 = blk * LBLK
        idx = np.arange(l0 - (W - 1), l0 + LBLK)
        valid = idx >= 0
        idxc = np.clip(idx, 0, L - 1)
        t_win = np.where(valid, times[b, idxc], 0.0).astype(np.float32)
        feat_win = np.where(valid[:, None], features[b, idxc, :], 0.0)
        tiw = (true_ids[b, idxc] & valid).astype(np.float32)
        t_row = times[b, l0:l0 + LBLK].astype(np.float32)
        rv = (np.arange(l0, l0 + LBLK) <=
              (sim + 1) * (int(lengths[b]) - 1)).astype(np.float32)

        dtpk = np.zeros((2, W2 + LBLK), np.float32)
        dtpk[0, :W2] = 1.0
        dtpk[1, :WIN] = t_win
        dtpk[0, W2:] = t_row
        dtpk[1, W2:] = -1.0

        fw = np.zeros((CH, W2 + NF), np.float32)
        fw[:, :WIN] = feat_win.T
        fw[:, W2:W2 + NF] = w2p

        if raw:
            par = np.zeros((128, 21 + LBLK), np.float32)
        else:
            par = np.zeros((128, NPAR), np.float32)
        par[:, 0] = tiw[:128]
        par[:LO, 1] = tiw[128:]
        par[:, 2] = rv
        par[:, 3:3 + HID] = w1[None, :]
        par[:, 3 + HID:3 + 2 * HID] = b1[None, :]
        if raw:
            par[:, 19] = t_win[:128]
            par[:LO, 20] = t_win[128:]
            par[:, 21:] = t_row[None, :]
            in_maps.append({"fw": fw, "par": par})
        else:
            in_maps.append({"dtpk": dtpk, "fw": fw, "par": par})

    res = run_bass_kernel_spmd(nc, in_maps, core_ids=list(range(NCORES)),
                               trace=TRACE)
    LAST = res

    out = np.zeros((BS, L, CH), np.float32)
    for core in range(NCORES):
        b, blk = divmod(core, NBLK)
        out[b, blk * LBLK:(blk + 1) * LBLK, :] = res.results[core]["out"]
    return out

